# revision 1
# baseline (speedup 1.0000x reference)
"""Trainium2 Bass kernel for nn_MinibatchDiscriminator (N=512, INSIZE=512, K=64, D=16).

Strategy (8 NeuronCores, SPMD, no collectives):
  out = concat(x, o_b) where o_b[i,k] = sum_j exp(-sum_d |feat[i,k,d]-feat[j,k,d]|)
  with the self term (j==i) removed; feat = x @ W.T + b.

Per core (row-shard of 64 i's, full j range):
  Phase 0 (replicated): load x, W; PE-transpose to get xT/wT; compute
    featT = (W @ x.T + b) as 8 chunks [128=(8k x 16d), 512=j] in float32r;
    fsT = same for this core's 64 rows (from the xs input — bitwise-identical
    math so the diagonal cancels exactly); R[j,k] = sum_d featT (PE ones-matmul)
    assembled as B[(s,k), j]; bias/scale tables for the exp.
  Phase 1: for each pair of rows (2 i's per PSUM tile):
    - per chunk: DVE computes min(featT_c, featT_c[:,i]) (single-op tensor_scalar,
      float32r out) or ACT computes |featT_c - featT_c[:,i]| (activation Abs);
      |a-b| = a + b - 2*min(a,b) so PE-accumulated mins + rank-1 terms give the
      L1 norm for min-chunks, direct sums for abs-chunks.
    - 16 matmuls with block-diag ones lhsT reduce d (16->1) into PSUM
      [128=(2i x 64k), 512=j]; a 17th matmul adds -B/2 on min-chunk rows.
    - One ACT activation computes exp(scale*psum + bias) with per-partition
      scale (+2 on min rows, -1 on abs rows) and bias (B_r-2R on min rows, 0 on
      abs rows), accumulating the row sum over j directly (accum_out).
  o_b = accum - 1; PE-transpose to [i, k] layout; DMA out.
"""
import sys

import numpy as np

sys.path.insert(0, "/opt/trn_rl_repo")

import concourse.bass as bass
import concourse.tile as tile
from concourse import bacc, mybir
from concourse.bass_utils import run_bass_kernel_spmd

AF = mybir.ActivationFunctionType
OP = mybir.AluOpType
FP32 = mybir.dt.float32
FP32R = mybir.dt.float32r

N, INSIZE, K, D = 512, 512, 64, 16
KD = K * D
NCORES = 8
NL = N // NCORES  # 64 rows per core
P = 128
CH = KD // P      # 8 chunks of (8 k x 16 d)
NT = INSIZE // P  # 4 contraction tiles
NG = NL // 2      # 32 groups of 2 rows
ACT_SC = {(0, 2), (0, 6), (1, 2), (1, 5), (1, 6)}  # (s, c) absdiffs on ScalarE

TRACE = False
_cache = {}


def _build():
    nc = bacc.Bacc("TRN2", target_bir_lowering=False)
    x_h = nc.dram_tensor("x", [N, INSIZE], FP32, kind="ExternalInput").ap()
    w_h = nc.dram_tensor("w", [KD, INSIZE], FP32, kind="ExternalInput").ap()
    b_h = nc.dram_tensor("bmat", [P, CH], FP32, kind="ExternalInput").ap()
    xs_h = nc.dram_tensor("xs", [NL, INSIZE], FP32, kind="ExternalInput").ap()
    ones_h = nc.dram_tensor("ones", [P, 16 * P], FP32R, kind="ExternalInput").ap()
    half_h = nc.dram_tensor("half", [P, P], FP32R, kind="ExternalInput").ap()
    iden_h = nc.dram_tensor("iden", [P, P], FP32, kind="ExternalInput").ap()
    scol_h = nc.dram_tensor("scol", [P, 1], FP32, kind="ExternalInput").ap()
    rmask_h = nc.dram_tensor("rmask", [P, 1], FP32, kind="ExternalInput").ap()
    elhs_h = nc.dram_tensor("elhs", [P, NG * P], FP32R, kind="ExternalInput").ap()
    erhs_h = nc.dram_tensor("erhs", [P, N], FP32R, kind="ExternalInput").ap()
    out_h = nc.dram_tensor("out", [NL, INSIZE + K], FP32, kind="ExternalOutput").ap()

    with tile.TileContext(nc) as tc:
        with (
            tc.tile_pool(name="const", bufs=1) as cst,
            tc.tile_pool(name="inp", bufs=1) as inp,
            tc.tile_pool(name="feat", bufs=1) as fpl,
            tc.tile_pool(name="ad", bufs=8) as adp,
            tc.tile_pool(name="cb", bufs=2) as cbp,
            tc.tile_pool(name="tp", bufs=2, space="PSUM") as tpp,
            tc.tile_pool(name="mm", bufs=2, space="PSUM") as mmp,
            tc.tile_pool(name="fs", bufs=1, space="PSUM") as fsp,
            tc.tile_pool(name="nm", bufs=3, space="PSUM") as nmp,
        ):
            # ---------------- loads ----------------
            # x/w first: everything downstream (transposes, featT) waits on them;
            # constants are only needed later.
            xs_sb = cst.tile([NL, INSIZE], FP32, tag="xs")
            nc.sync.dma_start(out=xs_sb, in_=xs_h)
            iden_sb = cst.tile([P, P], FP32, tag="iden")
            nc.sync.dma_start(out=iden_sb, in_=iden_h)
            b_sb = cst.tile([P, CH], FP32, tag="b")
            nc.sync.dma_start(out=b_sb, in_=b_h)
            x_sb = []
            for u in range(NT):
                t = inp.tile([P, INSIZE], FP32, tag=f"x{u}")
                nc.sync.dma_start(out=t, in_=x_h[P * u : P * (u + 1), :])
                x_sb.append(t)
            w_sb = []
            for c in range(CH):
                t = inp.tile([P, INSIZE], FP32, tag=f"w{c}")
                nc.scalar.dma_start(out=t, in_=w_h[P * c : P * (c + 1), :])
                w_sb.append(t)
            ones_sb = cst.tile([P, 16 * P], FP32R, tag="ones")
            nc.scalar.dma_start(out=ones_sb, in_=ones_h)
            half_sb = cst.tile([P, P], FP32R, tag="half")
            nc.scalar.dma_start(out=half_sb, in_=half_h)
            scol_sb = cst.tile([P, 1], FP32, tag="scol")
            nc.sync.dma_start(out=scol_sb, in_=scol_h)
            rmask_sb = cst.tile([P, 1], FP32, tag="rmask")
            nc.sync.dma_start(out=rmask_sb, in_=rmask_h)
            elhs_sb = cst.tile([P, NG * P], FP32R, tag="elhs")
            nc.scalar.dma_start(out=elhs_sb, in_=elhs_h)
            erhs_sb = cst.tile([P, N], FP32R, tag="erhs")
            nc.sync.dma_start(out=erhs_sb, in_=erhs_h)

            # x passthrough
            nc.sync.dma_start(out=out_h[:, 0:INSIZE], in_=xs_h)

            # ---------------- transposes ----------------
            # xsT first (needed by every fsT chunk), then xT, then per-chunk
            # W transposes feeding featT/fsT so chunk 0 unblocks early.
            ps = tpp.tile([P, N], FP32, tag="tp")
            for t in range(NT):
                nc.tensor.transpose(
                    ps[:, NL * t : NL * (t + 1)],
                    xs_sb[:, P * t : P * (t + 1)],
                    iden_sb[0:NL, 0:NL],
                )
            xsT_sb = cst.tile([P, NT * NL], FP32R, tag="xsT")
            nc.vector.tensor_copy(xsT_sb, ps[:, 0 : NT * NL])
            xT = []
            for t in range(NT):
                ps = tpp.tile([P, N], FP32, tag="tp")
                for u in range(NT):
                    nc.tensor.transpose(
                        ps[:, P * u : P * (u + 1)],
                        x_sb[u][:, P * t : P * (t + 1)],
                        iden_sb,
                    )
                sb = inp.tile([P, N], FP32R, tag=f"xT{t}")
                (nc.scalar.copy if t % 2 else nc.vector.tensor_copy)(sb, ps)
                xT.append(sb)

            # ---------------- featT / fsT per chunk (f32r) ----------------
            featT = []
            fsT_sb = cst.tile([P, CH * NL], FP32R, tag="fsT")
            for c in range(CH):
                ps = tpp.tile([P, N], FP32, tag="tp")
                for t in range(NT):
                    nc.tensor.transpose(
                        ps[:, P * t : P * (t + 1)],
                        w_sb[c][:, P * t : P * (t + 1)],
                        iden_sb,
                    )
                wTC = inp.tile([P, NT, P], FP32R, tag=f"wT{c}")
                for t in range(NT):
                    (nc.scalar.copy if t % 2 else nc.vector.tensor_copy)(
                        wTC[:, t, :], ps[:, P * t : P * (t + 1)]
                    )
                psf = mmp.tile([P, N], FP32, tag="mm")
                for t in range(NT):
                    nc.tensor.matmul(
                        psf, wTC[:, t, :], xT[t],
                        start=(t == 0), stop=(t == NT - 1),
                    )
                sb = fpl.tile([P, N], FP32R, tag=f"feat{c}")
                nc.vector.tensor_scalar(sb, psf, b_sb[:, c : c + 1], None, op0=OP.add)
                featT.append(sb)
                psf2 = fsp.tile([P, NL], FP32, tag="fs")
                for t in range(NT):
                    nc.tensor.matmul(
                        psf2, wTC[:, t, :], xsT_sb[:, NL * t : NL * (t + 1)],
                        start=(t == 0), stop=(t == NT - 1),
                    )
                nc.vector.tensor_scalar(
                    fsT_sb[:, NL * c : NL * (c + 1)], psf2, b_sb[:, c : c + 1],
                    None, op0=OP.add,
                )

            # ---------------- R / B / bias tables ----------------
            # B[(s,k), j] = R[j,k] = sum_d featT_r[(k,d), j]; both s-halves via
            # all 16 block-diag ones matrices accumulating into one PSUM tile.
            psr = tpp.tile([P, N], FP32, tag="tp")
            for s in range(2):
                for c in range(CH):
                    tsel = s * 8 + c
                    nc.tensor.matmul(
                        psr,
                        ones_sb[:, P * tsel : P * (tsel + 1)],
                        featT[c],
                        start=(tsel == 0),
                        stop=(tsel == 15),
                    )
            B_sb = cst.tile([P, N], FP32, tag="B")
            nc.vector.tensor_copy(B_sb, psr)
            B_r = cst.tile([P, N], FP32R, tag="Br")
            nc.vector.tensor_copy(B_r, B_sb)

            # Rs[(s,k), i] = R at this core's own rows (core-local, from fsT),
            # via the same 16 ones matrices so the diagonal cancels bitwise.
            psrs = fsp.tile([P, NL], FP32, tag="fs")
            for s in range(2):
                for c in range(CH):
                    tsel = s * 8 + c
                    nc.tensor.matmul(
                        psrs,
                        ones_sb[:, P * tsel : P * (tsel + 1)],
                        fsT_sb[:, NL * c : NL * (c + 1)],
                        start=(tsel == 0),
                        stop=(tsel == 15),
                    )
            Rs_sb = cst.tile([P, NL], FP32, tag="Rs")
            nc.vector.tensor_copy(Rs_sb, psrs)
            Rs_r = cst.tile([P, NL], FP32R, tag="Rsr")
            nc.vector.tensor_copy(Rs_r, Rs_sb)
            # Pass B: replicate phase-1's exact PSUM arithmetic at the diagonal:
            # v[p,i] = fp32(Rs - 0.5*Rs_r) via the same 16 ones-matmuls plus the
            # half matmul, in the same order. bias = -2*v (exact) makes the
            # exp argument exactly zero at j==i on min-chunk rows.
            psv = fsp.tile([P, NL], FP32, tag="fs")
            for s in range(2):
                for c in range(CH):
                    tsel = s * 8 + c
                    nc.tensor.matmul(
                        psv,
                        ones_sb[:, P * tsel : P * (tsel + 1)],
                        fsT_sb[:, NL * c : NL * (c + 1)],
                        start=(tsel == 0),
                        stop=False,
                    )
            nc.tensor.matmul(psv, half_sb, Rs_r, start=False, stop=True)
            Vs_sb = cst.tile([P, NL], FP32, tag="Vs")
            nc.vector.tensor_copy(Vs_sb, psv)
            # negA[p, g] = rmask * (-2*Vs)[p, 2g + (p >= 64)]
            negA = cst.tile([P, NG], FP32, tag="negA")
            vv = Vs_sb.rearrange("p (g s) -> p s g", s=2)
            nc.vector.tensor_copy(negA[0:NL, :], vv[0:NL, 0, :])
            nc.vector.tensor_copy(negA[NL:P, :], vv[NL:P, 1, :])
            nc.vector.tensor_scalar(negA, negA, -2.0, None, op0=OP.mult)
            nc.vector.tensor_scalar(negA, negA, rmask_sb[:, 0:1], None, op0=OP.mult)

            o_sb = cst.tile([P, NG], FP32, tag="o")

            # ---------------- phase 1 ----------------
            for g in range(NG):
                psn = nmp.tile([P, N], FP32, tag="nm")
                for s in range(2):
                    i = 2 * g + s
                    for c in range(CH):
                        ad = adp.tile([P, N], FP32R, tag="ad")
                        col = fsT_sb[:, NL * c + i : NL * c + i + 1].bitcast(FP32)
                        fin = featT[c].bitcast(FP32)
                        if (s, c) in ACT_SC:
                            nc.scalar.activation(
                                ad, fin, AF.Abs, bias=col, scale=-1.0
                            )
                        else:
                            nc.vector.tensor_scalar(
                                ad, fin, col, None, op0=OP.min
                            )
                        tsel = s * 8 + c
                        nc.tensor.matmul(
                            psn,
                            ones_sb[:, P * tsel : P * (tsel + 1)],
                            ad,
                            start=(s == 0 and c == 0),
                            stop=False,
                        )
                nc.tensor.matmul(psn, half_sb, B_r, start=False, stop=False)
                nc.tensor.matmul(
                    psn, elhs_sb[:, P * g : P * (g + 1)], erhs_sb,
                    start=False, stop=True,
                )
                cb = cbp.tile([P, N], FP32, tag="cb")
                nc.scalar.activation(
                    cb, psn, AF.Exp,
                    bias=negA[:, g : g + 1],
                    scale=scol_sb[:, 0:1],
                    accum_out=o_sb[:, g : g + 1],
                )

            # ---------------- epilogue ----------------
            pso = tpp.tile([NG, P], FP32, tag="tp")
            nc.tensor.transpose(pso, o_sb, iden_sb)
            oT_sb = cst.tile([NG, P], FP32, tag="oT")
            nc.vector.tensor_copy(oT_sb, pso)
            out_ob = out_h[:, INSIZE : INSIZE + K].rearrange("(g s) k -> g s k", s=2)
            nc.sync.dma_start(out=out_ob, in_=oT_sb.rearrange("g (s k) -> g s k", s=2))

    nc.finalize()
    return nc


def _consts():
    ones = np.zeros((P, 16, P), np.float32)
    for s in range(2):
        for c in range(CH):
            tsel = s * 8 + c
            for gl in range(8):
                ones[16 * gl : 16 * (gl + 1), tsel, 64 * s + 8 * c + gl] = 1.0
    ones = np.ascontiguousarray(ones.reshape(P, 16 * P))
    # half: -0.5 * identity on min-chunk rows, 0 on abs-chunk rows
    half = np.zeros((P, P), np.float32)
    minrow = np.ones(P, bool)
    for s, c in ACT_SC:
        minrow[64 * s + 8 * c : 64 * s + 8 * c + 8] = False
    half[np.arange(P), np.arange(P)] = np.where(minrow, -0.5, 0.0)
    iden = np.eye(P, dtype=np.float32)
    scol = np.where(minrow, 2.0, -1.0).astype(np.float32).reshape(P, 1)
    rmask = minrow.astype(np.float32).reshape(P, 1)
    return ones, half, iden, scol, rmask, minrow


BIG = 200.0


def _eraser(core, minrow):
    # eras_lhs[q, g*128+p] = sign(p)*BIG at q = 64*core + 2g + (p//64) - 128*t*
    # eras_rhs[q, j] = 1 at j = q + 128*t*   (t* = core // 2)
    t_star = core // 2
    elhs = np.zeros((P, NG * P), np.float32)
    sign = np.where(minrow, -BIG, BIG)
    for g in range(NG):
        for p in range(P):
            q = 64 * core + 2 * g + (p // 64) - P * t_star
            elhs[q, g * P + p] = sign[p]
    erhs = np.zeros((P, N), np.float32)
    erhs[np.arange(P), np.arange(P) + P * t_star] = 1.0
    return elhs, erhs


def kernel(x, W, b):
    x = np.ascontiguousarray(np.asarray(x, np.float32))
    W = np.ascontiguousarray(np.asarray(W, np.float32))
    b = np.asarray(b, np.float32)
    if "nc" not in _cache:
        _cache["nc"] = _build()
    nc = _cache["nc"]
    ones, half, iden, scol, rmask, minrow = _consts()
    bmat = np.ascontiguousarray(b.reshape(CH, P).T)
    in_maps = []
    for c in range(NCORES):
        in_maps.append(
            {
                "x": x,
                "w": W,
                "bmat": bmat,
                "xs": np.ascontiguousarray(x[NL * c : NL * (c + 1)]),
                "ones": ones,
                "half": half,
                "iden": iden,
                "scol": scol,
                "rmask": rmask,
            }
        )
        elhs, erhs = _eraser(c, minrow)
        in_maps[-1]["elhs"] = elhs
        in_maps[-1]["erhs"] = erhs
    res = run_bass_kernel_spmd(
        nc, in_maps, core_ids=list(range(NCORES)), trace=TRACE
    )
    _cache["last_results"] = res
    return np.ascontiguousarray(
        np.concatenate([res.results[c]["out"] for c in range(NCORES)], axis=0)
    )

